# revision 1
# baseline (speedup 1.0000x reference)
"""Trainium2 Bass kernel for nn_MessageGNN (gnn_message_passing).

Sharding: destination-sharded edges across 8 cores.  Core k owns clauses
[k*50000,(k+1)*50000) and vars [k*12500,(k+1)*12500) and every edge whose
destination falls in its slice, so segment sums are fully core-local.

Per core, per edge type:
  - Edges are laid out window-major (1024 destinations per window); inside a
    window they are bucketed by 32768-row gather-table chunk (int16 index
    limit of dma_gather) and sorted by destination.
  - x^T tiles arrive feature-major straight from fp16 transpose-mode
    dma_gather.  Edge MLP: per 128-edge tile, stationary x^T / sat^T against
    moving weight chunks accumulate m[e,d] in PSUM; lrelu = ACT scale-copy +
    DVE max.  Segment-mean via one-hot matmul (one fused DVE op builds
    onehot * (1/cnt)) accumulated into two [128,512] PSUM window halves.
  - Node MLP fused per window: 4 weight-chunk matmuls (feats+bias / h / ctx /
    emb), with the tiny ctx gather folded into a host-computed projection
    ctx_emb @ W_ctx driven by a one-hot.  Outputs transposed back to
    row-major and DMA'd out.  Phase-3 partial sums (new nodes grouped by ctx)
    accumulate into a [128,64] tile per node type; the 64-row ctx update
    finishes on host.
"""

import sys
import threading

sys.path.insert(0, "/opt/trn_rl_repo")

import numpy as np

NV, NC, NU, E, D = 100000, 400000, 64, 1200000, 128
M = 8
CS, VS = NC // M, NV // M
WIN = 1024
CHUNK = 32768
PAD_DST = 1536.0
P = 128

F16 = np.float16
F32 = np.float32

_scale = 1  # test hook: shrink factor (1 = full problem)
_EDGE_ONLY = False  # debug: skip node phase
_NODE_ONLY = False  # debug: skip edge compute
_EDGE_LEVEL = 4  # debug: 1=gather 2=+mlp 3=+ohw 4=full


def _wrap_idx(vals):
    n = len(vals)
    arr = np.zeros((16, n // 16), np.int16)
    if n:
        arr[np.arange(n) % 16, np.arange(n) // 16] = vals
    return np.tile(arr, (8, 1))


def _prep_side(src, dst, sat, n_dst, slice_base, tab_rows, we):
    """Slot arrays + schedule for one edge type on one core.
    we: global per-destination 1/max(cnt,1) array."""
    mask = (dst >= slice_base) & (dst < slice_base + n_dst)
    es = np.nonzero(mask)[0]
    dstl = dst[es] - slice_base
    srcl = src[es]
    w_id = dstl // WIN
    c_id = srcl // CHUNK
    order = np.lexsort((dstl, c_id, w_id))
    es, dstl, srcl, w_id, c_id = (a[order] for a in (es, dstl, srcl, w_id, c_id))

    nwin = (n_dst + WIN - 1) // WIN
    nchunk = (tab_rows + CHUNK - 1) // CHUNK

    slot_src, slot_dstw, slot_e = [], [], []
    windows, idx_cols = [], []
    icol = 0
    for w in range(nwin):
        wsel = np.nonzero(w_id == w)[0]
        cw = c_id[wsel]
        gathers = []
        w_slot0 = len(slot_src)
        for c in range(nchunk):
            g = wsel[cw == c]
            n = len(g)
            if n == 0:
                continue
            npad = (-n) % P
            loc = (srcl[g] - c * CHUNK).tolist() + [0] * npad
            off = len(slot_src) - w_slot0
            slot_src.extend(loc)
            slot_dstw.extend((dstl[g] - w * WIN).tolist() + [-1] * npad)
            slot_e.extend(es[g].tolist() + [-1] * npad)
            ntot = n + npad
            idx_cols.append(_wrap_idx(np.asarray(loc, np.int64)))
            gathers.append(dict(chunk=c, icol=icol, n=ntot, off=off))
            icol += ntot // 16
        slots = len(slot_src) - w_slot0
        tiles = []
        dstw = np.asarray(slot_dstw[w_slot0:], np.int64)
        for t in range(slots // P):
            dv = dstw[t * P:(t + 1) * P]
            real = dv >= 0
            if not real.any():
                tiles.append(None)
                continue
            lo, hi = int(dv[real].min()), int(dv[real].max())
            base = (lo // 256) * 256
            width = ((hi + 1 - base + 255) // 256) * 256
            slices = []
            for q in range(width // 256):
                qlo = base + q * 256
                if ((dv[real] >= qlo) & (dv[real] < qlo + 256)).any():
                    slices.append((q, qlo // 512, qlo % 512))
            tiles.append(dict(base=base, width=width, slices=slices))
        windows.append(dict(slots=slots, gathers=gathers, tiles=tiles))

    S = len(slot_src)
    if S == 0:
        S = P
        slot_src, slot_dstw, slot_e = [0] * P, [-1] * P, [-1] * P
    dstw = np.asarray(slot_dstw, np.int64)
    eid = np.asarray(slot_e, np.int64)

    dst_rel = np.full(S, PAD_DST, F32)
    off = 0
    for wm in windows:
        for t, tm in enumerate(wm["tiles"]):
            if tm is None:
                continue
            sl = slice(off + t * P, off + (t + 1) * P)
            dv = dstw[sl]
            seg = dst_rel[sl]
            real = dv >= 0
            seg[real] = (dv[real] - tm["base"]).astype(F32)
            dst_rel[sl] = seg
        off += wm["slots"]

    real = eid >= 0
    wslot = np.zeros(S, F32)
    wslot[real] = we[dst[eid[real]]]
    satA = np.zeros((5, S), F16)
    satA[:4, real] = sat[eid[real]].T.astype(F16)
    satA[4, real] = 1.0

    idxA = (np.concatenate(idx_cols, axis=1) if idx_cols
            else np.zeros((P, 8), np.int16))
    return dict(
        idxA=idxA,
        dstA=np.ascontiguousarray(dst_rel.reshape(S // P, P).T.astype(F32)),
        wA=np.ascontiguousarray(wslot.reshape(S // P, P).T.astype(F16)),
        satA=satA,
        windows=windows,
        S=S,
    )


def _node_prep(feats, emb, ctx_ids, n_nodes):
    nwin = (n_nodes + WIN - 1) // WIN
    Np = nwin * WIN
    fT = np.zeros((feats.shape[1] + 1, Np), F16)
    fT[:-1, :n_nodes] = feats.T.astype(F16)
    fT[-1, :n_nodes] = 1.0
    eT = np.zeros((P, Np), F16)
    eT[:, :n_nodes] = emb.T.astype(F16)
    cx = np.full(Np, 300.0, F32)
    cx[:n_nodes] = ctx_ids.astype(F32)
    cxT = np.ascontiguousarray(cx.reshape(Np // P, P).T.astype(F16))
    return fT, eT, cxT, Np


def _build_core(meta):
    import concourse.mybir as mybir
    import concourse.tile as tile
    from concourse import bacc
    from concourse.masks import make_identity

    f16, f32, i16, i32 = (mybir.dt.float16, mybir.dt.float32,
                          mybir.dt.int16, mybir.dt.int32)
    cs, vs = meta["CS"], meta["VS"]

    nc = bacc.Bacc("TRN2", target_bir_lowering=False, debug=False, num_devices=1)
    io = {}

    def dram(name, shape, dt, kind="ExternalInput"):
        io[name] = nc.dram_tensor(name, list(shape), dt, kind=kind)
        return io[name]

    for side in ("A", "B"):
        tabrows = meta["tabrows"][side]
        dram(f"gtab{side}", [tabrows, D], f16)
        dram(f"idx{side}", meta[side]["idxA"].shape, i16)
        dram(f"dst{side}", meta[side]["dstA"].shape, f32)
        dram(f"w{side}", meta[side]["wA"].shape, f16)
        dram(f"sat{side}", meta[side]["satA"].shape, f16)
        dram(f"Wemb{side}", [P, D], f16)
        dram(f"Wsat{side}", [5, D], f16)
    for sd in ("C", "V"):
        Np = meta[f"Np{sd}"]
        dram(f"featsT{sd}", [17, Np], f16)
        dram(f"embT{sd}", [P, Np], f16)
        dram(f"ctx{sd}", [P, Np // P], f16)
        dram(f"Wf{sd}", [17, D], f16)
        dram(f"Wh{sd}", [P, D], f16)
        dram(f"We{sd}", [P, D], f16)
        dram(f"ctxproj{sd}", [64, D], f16)
    dram("outC", [cs, D], f32, kind="ExternalOutput")
    dram("outV", [vs, D], f32, kind="ExternalOutput")
    dram("accC", [P, 64], f32, kind="ExternalOutput")
    dram("accV", [P, 64], f32, kind="ExternalOutput")

    stage_max = max(
        max((w["slots"] for w in meta["A"]["windows"]), default=P),
        max((w["slots"] for w in meta["B"]["windows"]), default=P),
        P,
    )
    idx_max = max(
        max((g["n"] // 16 for w in meta["A"]["windows"] for g in w["gathers"]), default=8),
        max((g["n"] // 16 for w in meta["B"]["windows"] for g in w["gathers"]), default=8),
        8,
    )

    with tile.TileContext(nc) as tc:
        with tc.tile_pool(name="const", bufs=1) as cpool, \
             tc.tile_pool(name="stage", bufs=2) as spool, \
             tc.tile_pool(name="work", bufs=2) as wpool, \
             tc.tile_pool(name="hbuf", bufs=2) as hpool, \
             tc.tile_pool(name="psA", bufs=2, space="PSUM") as psA, \
             tc.tile_pool(name="psH", bufs=1, space="PSUM") as psH, \
             tc.tile_pool(name="psN", bufs=1, space="PSUM") as psN:

            ident = cpool.tile([P, P], f32)
            make_identity(nc, ident[:])
            iota_i = cpool.tile([P, WIN], i32)
            nc.gpsimd.iota(iota_i[:], pattern=[[1, WIN]], base=0, channel_multiplier=0)
            iota16 = cpool.tile([P, WIN], f16)
            nc.vector.tensor_copy(iota16[:], iota_i[:])
            iota64f = cpool.tile([P, 64], f32)
            nc.vector.tensor_copy(iota64f[:], iota_i[:, :64])
            z1 = cpool.tile([1, P], f16)
            nc.gpsimd.memset(z1[:], 0.0)
            z512 = cpool.tile([1, 512], f16)
            nc.gpsimd.memset(z512[:], 0.0)

            wt = {}
            for nm in ("WembA", "WsatA", "WembB", "WsatB",
                       "WfC", "WhC", "WeC", "ctxprojC",
                       "WfV", "WhV", "WeV", "ctxprojV"):
                t = cpool.tile(list(io[nm].shape), f16, tag=nm)
                nc.sync.dma_start(t[:], io[nm][:])
                wt[nm] = t

            acc_sb = {}
            for sd in ("C", "V"):
                a = cpool.tile([P, 64], f32, tag=f"acc{sd}")
                nc.vector.memset(a[:], 0.0)
                acc_sb[sd] = a

            for side, sd, n_nodes in (("A", "C", cs), ("B", "V", vs)):
                sm = meta[side]
                gtab = io[f"gtab{side}"]
                tabrows = meta["tabrows"][side]
                tile_off = 0
                for w, wm in enumerate(sm["windows"]):
                    slots = wm["slots"]
                    ntiles = slots // P
                    stage = spool.tile([P, 1, stage_max], f16, tag="stage")
                    for g in (wm["gathers"] if not _NODE_ONLY else []):
                        n = g["n"]
                        it = wpool.tile([P, idx_max], i16, tag="idx")
                        nc.sync.dma_start(
                            it[:, :n // 16],
                            io[f"idx{side}"][:, g["icol"]:g["icol"] + n // 16])
                        c0 = g["chunk"] * CHUNK
                        c1 = min(c0 + CHUNK, tabrows)
                        # >512-idx transpose gathers crash the exec unit;
                        # split into <=512-idx calls (wrap layout slices
                        # cleanly at 512 = 32 idx columns)
                        for o in range(0, n, 512):
                            ns = min(512, n - o)
                            nc.gpsimd.dma_gather(
                                out_ap=stage[:, :, g["off"] + o:g["off"] + o + ns],
                                in_ap=gtab[c0:c1, :],
                                idxs_ap=it[:, o // 16:o // 16 + ns // 16],
                                num_idxs=ns, num_idxs_reg=ns, elem_size=D,
                                transpose=True)
                    if ntiles:
                        dstt = wpool.tile([P, max(ntiles, 1)], f32, tag="dstt")
                        nc.sync.dma_start(dstt[:, :ntiles],
                                          io[f"dst{side}"][:, tile_off:tile_off + ntiles])
                        wtt = wpool.tile([P, max(ntiles, 1)], f16, tag="wtt")
                        nc.sync.dma_start(wtt[:, :ntiles],
                                          io[f"w{side}"][:, tile_off:tile_off + ntiles])
                        satt = wpool.tile([5, stage_max], f16, tag="satt")
                        nc.sync.dma_start(
                            satt[:, :slots],
                            io[f"sat{side}"][:, tile_off * P:tile_off * P + slots])
                    hps = [psH.tile([P, 512], f32, tag=f"h{i}", name=f"hps{i}")
                           for i in range(2)]
                    for i in range(2):
                        nc.tensor.matmul(hps[i][:], lhsT=z1[:], rhs=z512[:],
                                         start=True, stop=False,
                                         skip_group_check=True)
                    for t in range(ntiles if not _NODE_ONLY and _EDGE_LEVEL >= 2 else 0):
                        tm = wm["tiles"][t]
                        mps = psA.tile([P, P], f32, tag="mps")
                        nc.tensor.matmul(mps[:], lhsT=stage[:, 0, t * P:(t + 1) * P],
                                         rhs=wt[f"Wemb{side}"][:], start=True, stop=False)
                        nc.tensor.matmul(mps[:], lhsT=satt[:, t * P:(t + 1) * P],
                                         rhs=wt[f"Wsat{side}"][:], start=False, stop=True)
                        tmp = wpool.tile([P, P], f32, tag="lrtmp")
                        nc.scalar.activation(tmp[:], mps[:],
                                             mybir.ActivationFunctionType.Copy, scale=0.1)
                        msb = wpool.tile([P, P], f16, tag="msb")
                        nc.vector.tensor_tensor(out=msb[:], in0=mps[:], in1=tmp[:],
                                                op=mybir.AluOpType.max)
                        if tm is None or _EDGE_LEVEL < 3:
                            continue
                        wd = tm["width"]
                        ohw = wpool.tile([P, WIN], f16, tag="ohw")
                        nc.vector.scalar_tensor_tensor(
                            out=ohw[:, :wd], in0=iota16[:, :wd],
                            scalar=dstt[:, t:t + 1],
                            in1=wtt[:, t:t + 1].to_broadcast([P, wd]),
                            op0=mybir.AluOpType.is_equal, op1=mybir.AluOpType.mult)
                        for (q, half, col) in (tm["slices"] if _EDGE_LEVEL >= 4 else []):
                            nc.tensor.matmul(hps[half][:, col:col + 256],
                                             lhsT=msb[:], rhs=ohw[:, q * 256:q * 256 + 256],
                                             start=False, stop=True, skip_group_check=True)
                    tile_off += ntiles
                    hT = hpool.tile([P, WIN], f16, tag="hT")
                    nc.vector.tensor_copy(hT[:, :512], hps[0][:])
                    nc.vector.tensor_copy(hT[:, 512:], hps[1][:])

                    # ---- node phase for this window (WIN nodes, padded) ----
                    for g0 in ((0, 512) if not _EDGE_ONLY else ()):
                        cga = w * WIN + g0
                        ctx16 = wpool.tile([P, 4], f16, tag="ctx16")
                        nc.sync.dma_start(ctx16[:], io[f"ctx{sd}"][:, cga // P:cga // P + 4])
                        ctx32 = wpool.tile([P, 4], f32, tag="ctx32")
                        nc.vector.tensor_copy(ctx32[:], ctx16[:])
                        featsl = wpool.tile([17, 512], f16, tag="featsl")
                        nc.sync.dma_start(featsl[:], io[f"featsT{sd}"][:, cga:cga + 512])
                        embl = wpool.tile([P, 512], f16, tag="embl")
                        nc.sync.dma_start(embl[:], io[f"embT{sd}"][:, cga:cga + 512])
                        ohuT = wpool.tile([64, 512], f16, tag="ohuT")
                        ohu_f = []
                        for j in range(4):
                            ohuf = wpool.tile([P, 64], f32, tag=f"ohuf{j}")
                            nc.vector.tensor_single_scalar(
                                out=ohuf[:], in_=iota64f[:],
                                scalar=ctx32[:, j:j + 1], op=mybir.AluOpType.is_equal)
                            ohu_f.append(ohuf)
                            tps = psA.tile([P, P], f32, tag="tp")
                            nc.tensor.matmul(tps[:64, :], lhsT=ohuf[:], rhs=ident[:],
                                             is_transpose=True, skip_group_check=True)
                            nc.vector.tensor_copy(ohuT[:, j * P:(j + 1) * P], tps[:64, :])
                        nps = psN.tile([P, 512], f32, tag="nps")
                        nc.tensor.matmul(nps[:], lhsT=wt[f"Wf{sd}"][:],
                                         rhs=featsl[:], start=True, stop=False)
                        nc.tensor.matmul(nps[:], lhsT=wt[f"Wh{sd}"][:],
                                         rhs=hT[:, g0:g0 + 512], start=False, stop=False)
                        nc.tensor.matmul(nps[:], lhsT=wt[f"ctxproj{sd}"][:],
                                         rhs=ohuT[:], start=False, stop=False)
                        nc.tensor.matmul(nps[:], lhsT=wt[f"We{sd}"][:],
                                         rhs=embl[:], start=False, stop=True)
                        ntmp = wpool.tile([P, 512], f32, tag="ntmp")
                        nc.scalar.activation(ntmp[:], nps[:],
                                             mybir.ActivationFunctionType.Copy, scale=0.1)
                        nsb = wpool.tile([P, 512], f32, tag="nsb")
                        nc.vector.tensor_tensor(out=nsb[:], in0=nps[:], in1=ntmp[:],
                                                op=mybir.AluOpType.max)
                        aps = psN.tile([P, 64], f32, tag="aps")
                        for j in range(4):
                            rows = min(P, max(0, n_nodes - (cga + j * P)))
                            tps2 = psA.tile([P, P], f32, tag="tp")
                            nc.tensor.matmul(tps2[:], lhsT=nsb[:, j * P:(j + 1) * P],
                                             rhs=ident[:], is_transpose=True,
                                             skip_group_check=True)
                            osb = wpool.tile([P, P], f32, tag="osb")
                            nc.vector.tensor_copy(osb[:], tps2[:])
                            if rows > 0:
                                out_t = io["outC"] if sd == "C" else io["outV"]
                                nc.sync.dma_start(
                                    out_t[cga + j * P:cga + j * P + rows, :],
                                    osb[:rows, :])
                            nc.tensor.matmul(aps[:], lhsT=osb[:], rhs=ohu_f[j][:],
                                             start=(j == 0), stop=(j == 3))
                        nc.vector.tensor_add(acc_sb[sd][:], acc_sb[sd][:], aps[:])

            nc.sync.dma_start(io["accC"][:], acc_sb["C"][:])
            nc.sync.dma_start(io["accV"][:], acc_sb["V"][:])
    nc.compile()
    return nc


def _run_cores(ncs, in_maps):
    """Compile + dispatch one program per NeuronCore, concurrently."""
    import jax
    from concourse import bass2jax
    from concourse.bass2jax import _bass_exec_p, install_neuronx_cc_hook
    import concourse.mybir as mybir

    install_neuronx_cc_hook()
    devs = jax.devices()[:len(ncs)]
    pending = []
    for i, nc in enumerate(ncs):
        in_names, out_names, out_avals, zero_outs = [], [], [], []
        for alloc in nc.m.functions[0].allocations:
            if not isinstance(alloc, mybir.MemoryLocationSet):
                continue
            name = alloc.memorylocations[0].name
            if alloc.kind == "ExternalInput":
                in_names.append(name)
            elif alloc.kind == "ExternalOutput":
                shape = tuple(alloc.tensor_shape)
                dtype = mybir.dt.np(alloc.dtype)
                out_names.append(name)
                out_avals.append(jax.core.ShapedArray(shape, dtype))
                zero_outs.append(np.zeros(shape, dtype))
        n_params = len(in_names)
        all_names = in_names + out_names

        def _body(*args, _oa=tuple(out_avals), _an=tuple(all_names),
                  _on=tuple(out_names), _nc=nc):
            return tuple(_bass_exec_p.bind(
                *args, out_avals=_oa, in_names=_an, out_names=_on,
                lowering_input_output_aliases=(),
                sim_require_finite=True, sim_require_nnan=True, nc=_nc,
            ))

        donate = tuple(range(n_params, n_params + len(out_names)))
        pid = np.zeros((1, 1), np.uint32)
        ins = [pid if n == "partition_id" else np.asarray(in_maps[i][n])
               for n in in_names]
        with jax.default_device(devs[i]):
            fn = jax.jit(_body, keep_unused=True)
            # compile (serial; axon compile path is not thread-safe) and
            # dispatch (async; all cores end up executing concurrently)
            outs = fn(*ins, *zero_outs)
        pending.append((out_names, outs))
        _timing_handles.append(dict(fn=fn, ins=ins, zeros=zero_outs,
                                    dev=devs[i], out_names=out_names))
    return [{n: np.asarray(o) for n, o in zip(on, outs)}
            for (on, outs) in pending]


_timing_handles = []


def kernel(**inputs):
    inp = {k: np.asarray(v) for k, v in inputs.items()}
    var_emb, clause_emb, ctx_emb = inp["var_emb"], inp["clause_emb"], inp["ctx_emb"]
    nv, ncl, nu = var_emb.shape[0], clause_emb.shape[0], ctx_emb.shape[0]
    cs, vs = ncl // M, nv // M

    W_vc, b_vc = inp["W_vc"].astype(F32), inp["b_vc"].astype(F32)
    W_cv, b_cv = inp["W_cv"].astype(F32), inp["b_cv"].astype(F32)
    W_c, b_c = inp["W_c"].astype(F32), inp["b_c"].astype(F32)
    W_v, b_v = inp["W_v"].astype(F32), inp["b_v"].astype(F32)

    a_src = inp["assigns_src"].astype(np.int64)
    a_dst = inp["assigns_dst"].astype(np.int64)
    c_src = inp["contains_src"].astype(np.int64)
    c_dst = inp["contains_dst"].astype(np.int64)
    var_ctx = inp["var_ctx"].astype(np.int64)
    clause_ctx = inp["clause_ctx"].astype(np.int64)

    cnt_c = np.bincount(a_dst, minlength=ncl).astype(F32)
    cnt_v = np.bincount(c_dst, minlength=nv).astype(F32)
    we_c = 1.0 / np.maximum(cnt_c, 1.0)
    we_v = 1.0 / np.maximum(cnt_v, 1.0)

    gtabA = var_emb.astype(F16)      # assigns gather var_emb
    gtabB = clause_emb.astype(F16)   # contains gathers clause_emb

    # edge MLP weight chunks (+bias row on the sat chunk)
    WembA = np.ascontiguousarray(W_vc[4:4 + D]).astype(F16)
    WsatA = np.vstack([W_vc[:4], b_vc[None, :]]).astype(F16)
    WembB = np.ascontiguousarray(W_cv[4:4 + D]).astype(F16)
    WsatB = np.vstack([W_cv[:4], b_cv[None, :]]).astype(F16)

    # node MLP chunks: rows [0:16 feats][16:144 h][144:272 ctx][272:400 emb]
    def node_w(Wn, bn):
        nf = Wn.shape[0] - 3 * D
        Wf = np.vstack([Wn[:nf], bn[None, :]]).astype(F16)
        Wh = np.ascontiguousarray(Wn[nf:nf + D]).astype(F16)
        ctxproj = (ctx_emb.astype(F32) @ Wn[nf + D:nf + 2 * D]).astype(F16)
        We = np.ascontiguousarray(Wn[nf + 2 * D:nf + 3 * D]).astype(F16)
        return Wf, Wh, ctxproj, We

    WfC, WhC, ctxprojC, WeC = node_w(W_c, b_c)
    WfV, WhV, ctxprojV, WeV = node_w(W_v, b_v)

    metas, in_maps = [], []
    for k in range(M):
        mA = _prep_side(a_src, a_dst, inp["edge_sat_vc"], cs, k * cs, nv, we_c)
        mB = _prep_side(c_src, c_dst, inp["edge_sat_cv"], vs, k * vs, ncl, we_v)
        fTC, eTC, cxC, NpC = _node_prep(inp["clause_feats"][k * cs:(k + 1) * cs],
                                        clause_emb[k * cs:(k + 1) * cs],
                                        clause_ctx[k * cs:(k + 1) * cs], cs)
        fTV, eTV, cxV, NpV = _node_prep(inp["var_feats"][k * vs:(k + 1) * vs],
                                        var_emb[k * vs:(k + 1) * vs],
                                        var_ctx[k * vs:(k + 1) * vs], vs)
        meta = dict(A=mA, B=mB, NpC=NpC, NpV=NpV, CS=cs, VS=vs,
                    tabrows=dict(A=nv, B=ncl))
        metas.append(meta)
        in_maps.append(dict(
            gtabA=gtabA, gtabB=gtabB,
            idxA=mA["idxA"], dstA=mA["dstA"], wA=mA["wA"], satA=mA["satA"],
            idxB=mB["idxA"], dstB=mB["dstA"], wB=mB["wA"], satB=mB["satA"],
            WembA=WembA, WsatA=WsatA, WembB=WembB, WsatB=WsatB,
            featsTC=fTC, embTC=eTC, ctxC=cxC,
            WfC=WfC, WhC=WhC, WeC=WeC, ctxprojC=ctxprojC,
            featsTV=fTV, embTV=eTV, ctxV=cxV,
            WfV=WfV, WhV=WhV, WeV=WeV, ctxprojV=ctxprojV,
        ))

    ncs = [_build_core(m) for m in metas]
    results = _run_cores(ncs, in_maps)

    new_clause = np.concatenate([r["outC"] for r in results], 0)
    new_var = np.concatenate([r["outV"] for r in results], 0)
    accC = np.sum([r["accC"] for r in results], 0)   # [128 d, 64 u]
    accV = np.sum([r["accV"] for r in results], 0)

    cnt_cu = np.bincount(clause_ctx, minlength=nu).astype(F32)
    cnt_vu = np.bincount(var_ctx, minlength=nu).astype(F32)
    c_ctx = (accC / np.maximum(cnt_cu, 1.0)[None, :]).T   # [64, 128]
    v_ctx = (accV / np.maximum(cnt_vu, 1.0)[None, :]).T
    zu = np.concatenate([inp["ctx_feats"].astype(F32), c_ctx, v_ctx,
                         ctx_emb.astype(F32)], 1) @ inp["W_u"].astype(F32) \
        + inp["b_u"].astype(F32)
    new_ctx = np.where(zu >= 0, zu, 0.1 * zu).astype(F32)

    return np.concatenate([new_clause, new_var, new_ctx], 0).astype(F32)



# revision 2
# speedup vs baseline: 7.7395x; 7.7395x over previous
"""Trainium2 Bass kernel for nn_MessageGNN (gnn_message_passing).

Sharding: destination-sharded edges across 8 cores.  Core k owns clauses
[k*50000,(k+1)*50000) and vars [k*12500,(k+1)*12500) and every edge whose
destination falls in its slice, so segment sums are fully core-local.

All 8 cores run ONE identical Bass program (SPMD) dispatched once via
shard_map — per-core variation lives entirely in the data.  The gather
schedule is made uniform by padding every (window, table-chunk) gather
group to the max count over the 8 cores (pad slots gather row 0, carry
zero sat/weight, dst sentinel 1536 so they contribute nothing).

Per core, per edge type:
  - Edges are laid out window-major (1024 destinations per window),
    bucketed by 32768-row gather-table chunk (int16 index limit of
    dma_gather) and sorted by destination.  x^T tiles arrive
    feature-major from fp16 transpose-mode dma_gather.
  - Edge MLP per 128-edge tile: stationary x^T / sat^T against moving
    weight chunks accumulate z[e,d] into a grouped PSUM tile; one Prelu
    (alpha=0.1) activation per 4-tile group does the leaky relu.
  - Segment-mean via one-hot matmul over the full 1024-dst window (one
    DVE op builds onehot * (1/cnt); two N=512 matmuls accumulate the
    window's h^T halves in PSUM).
  - Node MLP fused per window: 4 weight-chunk matmuls (feats+bias / h /
    ctx / emb) with the ctx gather folded into a host-computed
    projection driven by a one-hot.  Phase-3 partial sums accumulate
    into a [128,64] tile per node type; the 64-row ctx update finishes
    on host.
"""

import sys

sys.path.insert(0, "/opt/trn_rl_repo")

import numpy as np

NV, NC, NU, E, D = 100000, 400000, 64, 1200000, 128
M = 8
WIN = 1024
CHUNK = 32768
PAD_DST = 1536.0
P = 128
GRP = 4  # tiles per grouped-Prelu PSUM tile

F16 = np.float16
F32 = np.float32


def _wrap_idx(vals):
    n = len(vals)
    arr = np.zeros((16, n // 16), np.int16)
    if n:
        arr[np.arange(n) % 16, np.arange(n) // 16] = vals
    return np.tile(arr, (8, 1))


def _side_plan(src, dst, n_dst, tab_rows):
    """Uniform cross-core schedule for one edge type.

    Returns (sched, S, total_icols, percore) where sched is shared by all
    cores and percore[k] holds core k's sorted edge arrays."""
    nwin = (n_dst + WIN - 1) // WIN
    nchunk = (tab_rows + CHUNK - 1) // CHUNK
    counts = np.zeros((M, nwin, nchunk), np.int64)
    percore = []
    for k in range(M):
        base = k * n_dst
        mask = (dst >= base) & (dst < base + n_dst)
        es = np.nonzero(mask)[0]
        dstl = dst[es] - base
        srcl = src[es]
        w_id = dstl // WIN
        c_id = srcl // CHUNK
        order = np.lexsort((dstl, c_id, w_id))
        es, dstl, srcl, w_id, c_id = (a[order] for a in (es, dstl, srcl, w_id, c_id))
        np.add.at(counts[k], (w_id, c_id), 1)
        percore.append((es, dstl, srcl, w_id, c_id))
    npad = ((counts.max(0) + P - 1) // P) * P  # [nwin, nchunk]

    sched = []
    icol = 0
    for w in range(nwin):
        groups = []
        off = 0
        for c in range(nchunk):
            n = int(npad[w, c])
            if n == 0:
                continue
            groups.append(dict(chunk=c, n=n, off=off, icol=icol))
            off += n
            icol += n // 16
        sched.append(dict(slots=off, groups=groups))
    S = sum(wm["slots"] for wm in sched)
    if S == 0:
        sched[0] = dict(slots=P, groups=[dict(chunk=0, n=P, off=0, icol=0)])
        S, icol = P, P // 16
    return dict(sched=sched, S=S, icols=icol, nwin=nwin, nchunk=nchunk), percore


def _fill_side(plan, edges, sat, we, dst_glob):
    """Core-local slot arrays laid out per the shared schedule."""
    sched, S, icols, nchunk = plan["sched"], plan["S"], plan["icols"], plan["nchunk"]
    es, dstl, srcl, w_id, c_id = edges
    key = w_id * nchunk + c_id  # non-decreasing after the lexsort

    slot_src = np.zeros(S, np.int64)
    slot_dstw = np.full(S, -1, np.int64)
    slot_e = np.full(S, -1, np.int64)
    idxA = np.zeros((P, icols), np.int16)
    base = 0
    for w, wm in enumerate(sched):
        for g in wm["groups"]:
            c = g["chunk"]
            lo = np.searchsorted(key, w * nchunk + c, "left")
            hi = np.searchsorted(key, w * nchunk + c, "right")
            n = hi - lo
            s0 = base + g["off"]
            loc = np.zeros(g["n"], np.int64)
            loc[:n] = srcl[lo:hi] - c * CHUNK
            slot_src[s0:s0 + g["n"]] = loc
            slot_dstw[s0:s0 + n] = dstl[lo:hi] - w * WIN
            slot_e[s0:s0 + n] = es[lo:hi]
            idxA[:, g["icol"]:g["icol"] + g["n"] // 16] = _wrap_idx(loc)
        base += wm["slots"]

    dst_rel = np.where(slot_dstw >= 0, slot_dstw, int(PAD_DST)).astype(F32)
    real = slot_e >= 0
    wslot = np.zeros(S, F32)
    wslot[real] = we[dst_glob[slot_e[real]]]
    satA = np.zeros((5, S), F16)
    satA[:4, real] = sat[slot_e[real]].T.astype(F16)
    satA[4, real] = 1.0
    return dict(
        idxA=idxA,
        dstA=np.ascontiguousarray(dst_rel.reshape(S // P, P).T.astype(F32)),
        wA=np.ascontiguousarray(wslot.reshape(S // P, P).T.astype(F16)),
        satA=satA,
    )


def _node_prep(feats, emb, ctx_ids, n_nodes, nwin):
    Np = nwin * WIN
    fT = np.zeros((feats.shape[1] + 1, Np), F16)
    fT[:-1, :n_nodes] = feats.T.astype(F16)
    fT[-1, :n_nodes] = 1.0
    eT = np.zeros((P, Np), F16)
    eT[:, :n_nodes] = emb.T.astype(F16)
    cx = np.full(Np, 300.0, F32)
    cx[:n_nodes] = ctx_ids.astype(F32)
    cxT = np.ascontiguousarray(cx.reshape(Np // P, P).T.astype(F16))
    return fT, eT, cxT, Np


def _build_program(meta):
    import concourse.mybir as mybir
    import concourse.tile as tile
    from concourse import bacc
    from concourse.masks import make_identity

    f16, f32, i16, i32 = (mybir.dt.float16, mybir.dt.float32,
                          mybir.dt.int16, mybir.dt.int32)
    cs, vs = meta["CS"], meta["VS"]

    nc = bacc.Bacc("TRN2", target_bir_lowering=False, debug=False, num_devices=1)
    io = {}

    def dram(name, shape, dt, kind="ExternalInput"):
        io[name] = nc.dram_tensor(name, list(shape), dt, kind=kind)
        return io[name]

    for side in ("A", "B"):
        plan = meta[side]
        dram(f"gtab{side}", [meta["tabrows"][side], D], f16)
        dram(f"idx{side}", [P, plan["icols"]], i16)
        dram(f"dst{side}", [P, plan["S"] // P], f32)
        dram(f"w{side}", [P, plan["S"] // P], f16)
        dram(f"sat{side}", [5, plan["S"]], f16)
        dram(f"Wemb{side}", [P, D], f16)
        dram(f"Wsat{side}", [5, D], f16)
    for sd in ("C", "V"):
        Np = meta[f"Np{sd}"]
        dram(f"featsT{sd}", [17, Np], f16)
        dram(f"embT{sd}", [P, Np], f16)
        dram(f"ctx{sd}", [P, Np // P], f16)
        dram(f"Wf{sd}", [17, D], f16)
        dram(f"Wh{sd}", [P, D], f16)
        dram(f"We{sd}", [P, D], f16)
        dram(f"ctxproj{sd}", [64, D], f16)
    dram("outC", [cs, D], f32, kind="ExternalOutput")
    dram("outV", [vs, D], f32, kind="ExternalOutput")
    dram("accC", [P, 64], f32, kind="ExternalOutput")
    dram("accV", [P, 64], f32, kind="ExternalOutput")

    stage_max = max(
        max((wm["slots"] for wm in meta["A"]["sched"]), default=P),
        max((wm["slots"] for wm in meta["B"]["sched"]), default=P),
        P,
    )
    idx_max = max(
        max((g["n"] // 16 for plan in (meta["A"], meta["B"])
             for wm in plan["sched"] for g in wm["groups"]), default=8),
        8,
    )

    with tile.TileContext(nc) as tc:
        with tc.tile_pool(name="const", bufs=1) as cpool, \
             tc.tile_pool(name="stage", bufs=2) as spool, \
             tc.tile_pool(name="work", bufs=2) as wpool, \
             tc.tile_pool(name="hbuf", bufs=2) as hpool, \
             tc.tile_pool(name="psA", bufs=2, space="PSUM") as psA, \
             tc.tile_pool(name="psH", bufs=1, space="PSUM") as psH, \
             tc.tile_pool(name="psN", bufs=1, space="PSUM") as psN:

            ident = cpool.tile([P, P], f32)
            make_identity(nc, ident[:])
            iota_i = cpool.tile([P, WIN], i32)
            nc.gpsimd.iota(iota_i[:], pattern=[[1, WIN]], base=0, channel_multiplier=0)
            iota16 = cpool.tile([P, WIN], f16)
            nc.vector.tensor_copy(iota16[:], iota_i[:])
            iota64f = cpool.tile([P, 64], f32)
            nc.vector.tensor_copy(iota64f[:], iota_i[:, :64])
            z1 = cpool.tile([1, P], f16)
            nc.gpsimd.memset(z1[:], 0.0)
            z512 = cpool.tile([1, 512], f16)
            nc.gpsimd.memset(z512[:], 0.0)

            wt = {}
            for nm in ("WembA", "WsatA", "WembB", "WsatB",
                       "WfC", "WhC", "WeC", "ctxprojC",
                       "WfV", "WhV", "WeV", "ctxprojV"):
                t = cpool.tile(list(io[nm].shape), f16, tag=nm)
                nc.sync.dma_start(t[:], io[nm][:])
                wt[nm] = t

            acc_sb = {}
            for sd in ("C", "V"):
                a = cpool.tile([P, 64], f32, tag=f"acc{sd}")
                nc.vector.memset(a[:], 0.0)
                acc_sb[sd] = a

            for side, sd, n_nodes in (("A", "C", cs), ("B", "V", vs)):
                plan = meta[side]
                gtab = io[f"gtab{side}"]
                tabrows = meta["tabrows"][side]
                tile_off = 0
                for w, wm in enumerate(plan["sched"]):
                    slots = wm["slots"]
                    ntiles = slots // P
                    stage = spool.tile([P, 1, stage_max], f16, tag="stage")
                    for g in wm["groups"]:
                        n = g["n"]
                        it = wpool.tile([P, idx_max], i16, tag="idx")
                        nc.sync.dma_start(
                            it[:, :n // 16],
                            io[f"idx{side}"][:, g["icol"]:g["icol"] + n // 16])
                        c0 = g["chunk"] * CHUNK
                        c1 = min(c0 + CHUNK, tabrows)
                        # >512-idx transpose gathers crash the exec unit;
                        # split into <=512-idx calls (wrap layout slices
                        # cleanly at 512 = 32 idx columns)
                        for o in range(0, n, 512):
                            ns = min(512, n - o)
                            nc.gpsimd.dma_gather(
                                out_ap=stage[:, :, g["off"] + o:g["off"] + o + ns],
                                in_ap=gtab[c0:c1, :],
                                idxs_ap=it[:, o // 16:o // 16 + ns // 16],
                                num_idxs=ns, num_idxs_reg=ns, elem_size=D,
                                transpose=True)
                    if ntiles:
                        dstt = wpool.tile([P, max(ntiles, 1)], f32, tag="dstt")
                        nc.sync.dma_start(dstt[:, :ntiles],
                                          io[f"dst{side}"][:, tile_off:tile_off + ntiles])
                        wtt = wpool.tile([P, max(ntiles, 1)], f16, tag="wtt")
                        nc.sync.dma_start(wtt[:, :ntiles],
                                          io[f"w{side}"][:, tile_off:tile_off + ntiles])
                        satt = wpool.tile([5, stage_max], f16, tag="satt")
                        nc.sync.dma_start(
                            satt[:, :slots],
                            io[f"sat{side}"][:, tile_off * P:tile_off * P + slots])
                    hps = [psH.tile([P, 512], f32, tag=f"h{i}", name=f"hps{i}")
                           for i in range(2)]
                    for i in range(2):
                        nc.tensor.matmul(hps[i][:], lhsT=z1[:], rhs=z512[:],
                                         start=True, stop=False,
                                         skip_group_check=True)
                    for t0g in range(0, ntiles, GRP):
                        gn = min(GRP, ntiles - t0g)
                        mps = psA.tile([P, GRP * P], f32, tag="mps")
                        for j in range(gn):
                            t = t0g + j
                            nc.tensor.matmul(mps[:, j * P:(j + 1) * P],
                                             lhsT=stage[:, 0, t * P:(t + 1) * P],
                                             rhs=wt[f"Wemb{side}"][:],
                                             start=True, stop=False)
                            nc.tensor.matmul(mps[:, j * P:(j + 1) * P],
                                             lhsT=satt[:, t * P:(t + 1) * P],
                                             rhs=wt[f"Wsat{side}"][:],
                                             start=False, stop=True)
                        msb = wpool.tile([P, GRP * P], f16, tag="msb")
                        nc.scalar.activation(msb[:, :gn * P], mps[:, :gn * P],
                                             mybir.ActivationFunctionType.Prelu,
                                             alpha=0.1)
                        for j in range(gn):
                            t = t0g + j
                            ohw = wpool.tile([P, WIN], f16, tag="ohw")
                            nc.vector.scalar_tensor_tensor(
                                out=ohw[:], in0=iota16[:],
                                scalar=dstt[:, t:t + 1],
                                in1=wtt[:, t:t + 1].to_broadcast([P, WIN]),
                                op0=mybir.AluOpType.is_equal,
                                op1=mybir.AluOpType.mult)
                            for half in range(2):
                                nc.tensor.matmul(
                                    hps[half][:], lhsT=msb[:, j * P:(j + 1) * P],
                                    rhs=ohw[:, half * 512:(half + 1) * 512],
                                    start=False, stop=True, skip_group_check=True)
                    tile_off += ntiles
                    hT = hpool.tile([P, WIN], f16, tag="hT")
                    nc.vector.tensor_copy(hT[:, :512], hps[0][:])
                    nc.vector.tensor_copy(hT[:, 512:], hps[1][:])

                    # ---- node phase for this window (WIN nodes, padded) ----
                    for g0 in (0, 512):
                        cga = w * WIN + g0
                        ctx16 = wpool.tile([P, 4], f16, tag="ctx16")
                        nc.sync.dma_start(ctx16[:], io[f"ctx{sd}"][:, cga // P:cga // P + 4])
                        ctx32 = wpool.tile([P, 4], f32, tag="ctx32")
                        nc.vector.tensor_copy(ctx32[:], ctx16[:])
                        featsl = wpool.tile([17, 512], f16, tag="featsl")
                        nc.sync.dma_start(featsl[:], io[f"featsT{sd}"][:, cga:cga + 512])
                        embl = wpool.tile([P, 512], f16, tag="embl")
                        nc.sync.dma_start(embl[:], io[f"embT{sd}"][:, cga:cga + 512])
                        ohuT = wpool.tile([64, 512], f16, tag="ohuT")
                        ohu_f = []
                        for j in range(4):
                            ohuf = wpool.tile([P, 64], f32, tag=f"ohuf{j}")
                            nc.vector.tensor_single_scalar(
                                out=ohuf[:], in_=iota64f[:],
                                scalar=ctx32[:, j:j + 1], op=mybir.AluOpType.is_equal)
                            ohu_f.append(ohuf)
                            tps = psA.tile([P, P], f32, tag="tp")
                            nc.tensor.matmul(tps[:64, :], lhsT=ohuf[:], rhs=ident[:],
                                             is_transpose=True, skip_group_check=True)
                            nc.vector.tensor_copy(ohuT[:, j * P:(j + 1) * P], tps[:64, :])
                        nps = psN.tile([P, 512], f32, tag="nps")
                        nc.tensor.matmul(nps[:], lhsT=wt[f"Wf{sd}"][:],
                                         rhs=featsl[:], start=True, stop=False)
                        nc.tensor.matmul(nps[:], lhsT=wt[f"Wh{sd}"][:],
                                         rhs=hT[:, g0:g0 + 512], start=False, stop=False)
                        nc.tensor.matmul(nps[:], lhsT=wt[f"ctxproj{sd}"][:],
                                         rhs=ohuT[:], start=False, stop=False)
                        nc.tensor.matmul(nps[:], lhsT=wt[f"We{sd}"][:],
                                         rhs=embl[:], start=False, stop=True)
                        nsb = wpool.tile([P, 512], f32, tag="nsb")
                        nc.scalar.activation(nsb[:], nps[:],
                                             mybir.ActivationFunctionType.Prelu,
                                             alpha=0.1)
                        aps = psN.tile([P, 64], f32, tag="aps")
                        for j in range(4):
                            rows = min(P, max(0, n_nodes - (cga + j * P)))
                            tps2 = psA.tile([P, P], f32, tag="tp")
                            nc.tensor.matmul(tps2[:], lhsT=nsb[:, j * P:(j + 1) * P],
                                             rhs=ident[:], is_transpose=True,
                                             skip_group_check=True)
                            osb = wpool.tile([P, P], f32, tag="osb")
                            nc.vector.tensor_copy(osb[:], tps2[:])
                            if rows > 0:
                                out_t = io["outC"] if sd == "C" else io["outV"]
                                nc.sync.dma_start(
                                    out_t[cga + j * P:cga + j * P + rows, :],
                                    osb[:rows, :])
                            nc.tensor.matmul(aps[:], lhsT=osb[:], rhs=ohu_f[j][:],
                                             start=(j == 0), stop=(j == 3))
                        nc.vector.tensor_add(acc_sb[sd][:], acc_sb[sd][:], aps[:])

            nc.sync.dma_start(io["accC"][:], acc_sb["C"][:])
            nc.sync.dma_start(io["accV"][:], acc_sb["V"][:])
    nc.compile()
    return nc


_spmd_state = {}

REPLICATED = ("gtabA", "gtabB", "WembA", "WsatA", "WembB", "WsatB",
              "WfC", "WhC", "WeC", "ctxprojC", "WfV", "WhV", "WeV", "ctxprojV")


def _run_spmd(nc, per_core_maps, repl_map):
    """One shard_map dispatch running the identical program on all 8 cores."""
    import concourse.mybir as mybir
    import jax
    from concourse.bass2jax import (_bass_exec_p, install_neuronx_cc_hook,
                                    partition_id_tensor)
    from jax.experimental.shard_map import shard_map
    from jax.sharding import Mesh, NamedSharding, PartitionSpec

    install_neuronx_cc_hook()
    partition_name = nc.partition_id_tensor.name if nc.partition_id_tensor else None
    in_names, out_names, out_avals, zero_shapes = [], [], [], []
    for alloc in nc.m.functions[0].allocations:
        if not isinstance(alloc, mybir.MemoryLocationSet):
            continue
        name = alloc.memorylocations[0].name
        if alloc.kind == "ExternalInput":
            if name != partition_name:
                in_names.append(name)
        elif alloc.kind == "ExternalOutput":
            shape = tuple(alloc.tensor_shape)
            dtype = mybir.dt.np(alloc.dtype)
            out_names.append(name)
            out_avals.append(jax.core.ShapedArray(shape, dtype))
            zero_shapes.append((shape, dtype))
    n_params = len(in_names)
    n_outs = len(out_names)
    all_names = list(in_names) + list(out_names)
    if partition_name is not None:
        all_names.append(partition_name)
    donate = tuple(range(n_params, n_params + n_outs))

    def _body(*args):
        operands = list(args)
        if partition_name is not None:
            operands.append(partition_id_tensor())
        return tuple(_bass_exec_p.bind(
            *operands, out_avals=tuple(out_avals), in_names=tuple(all_names),
            out_names=tuple(out_names), lowering_input_output_aliases=(),
            sim_require_finite=True, sim_require_nnan=True, nc=nc))

    devices = jax.devices()[:M]
    mesh = Mesh(np.asarray(devices), ("core",))
    in_specs = tuple(
        PartitionSpec() if nm in REPLICATED else PartitionSpec("core")
        for nm in in_names) + (PartitionSpec("core"),) * n_outs
    out_specs = (PartitionSpec("core"),) * n_outs
    sharded = jax.jit(
        shard_map(_body, mesh=mesh, in_specs=in_specs, out_specs=out_specs,
                  check_rep=False),
        donate_argnums=donate, keep_unused=True)

    sh_core = NamedSharding(mesh, PartitionSpec("core"))
    sh_repl = NamedSharding(mesh, PartitionSpec())
    d_ins = []
    for nm in in_names:
        if nm in REPLICATED:
            d_ins.append(jax.device_put(repl_map[nm], sh_repl))
        else:
            d_ins.append(jax.device_put(
                np.concatenate([m[nm] for m in per_core_maps], 0), sh_core))
    d_zeros = [jax.device_put(np.zeros((M * s[0], *s[1:]), dt), sh_core)
               for (s, dt) in zero_shapes]

    outs = sharded(*d_ins, *d_zeros)
    host = {nm: np.asarray(o) for nm, o in zip(out_names, outs)}

    _spmd_state.clear()
    _spmd_state.update(fn=sharded, d_ins=d_ins, outs=list(outs),
                       out_names=out_names)
    return host


def kernel(**inputs):
    inp = {k: np.asarray(v) for k, v in inputs.items()}
    var_emb, clause_emb, ctx_emb = inp["var_emb"], inp["clause_emb"], inp["ctx_emb"]
    nv, ncl, nu = var_emb.shape[0], clause_emb.shape[0], ctx_emb.shape[0]
    cs, vs = ncl // M, nv // M

    W_vc, b_vc = inp["W_vc"].astype(F32), inp["b_vc"].astype(F32)
    W_cv, b_cv = inp["W_cv"].astype(F32), inp["b_cv"].astype(F32)
    W_c, b_c = inp["W_c"].astype(F32), inp["b_c"].astype(F32)
    W_v, b_v = inp["W_v"].astype(F32), inp["b_v"].astype(F32)

    a_src = inp["assigns_src"].astype(np.int64)
    a_dst = inp["assigns_dst"].astype(np.int64)
    c_src = inp["contains_src"].astype(np.int64)
    c_dst = inp["contains_dst"].astype(np.int64)
    var_ctx = inp["var_ctx"].astype(np.int64)
    clause_ctx = inp["clause_ctx"].astype(np.int64)

    cnt_c = np.bincount(a_dst, minlength=ncl).astype(F32)
    cnt_v = np.bincount(c_dst, minlength=nv).astype(F32)
    we_c = 1.0 / np.maximum(cnt_c, 1.0)
    we_v = 1.0 / np.maximum(cnt_v, 1.0)

    gtabA = var_emb.astype(F16)      # assigns gathers var_emb
    gtabB = clause_emb.astype(F16)   # contains gathers clause_emb

    # edge MLP weight chunks (+bias row on the sat chunk)
    WembA = np.ascontiguousarray(W_vc[4:4 + D]).astype(F16)
    WsatA = np.vstack([W_vc[:4], b_vc[None, :]]).astype(F16)
    WembB = np.ascontiguousarray(W_cv[4:4 + D]).astype(F16)
    WsatB = np.vstack([W_cv[:4], b_cv[None, :]]).astype(F16)

    # node MLP chunks: rows [0:16 feats][16:144 h][144:272 ctx][272:400 emb]
    def node_w(Wn, bn):
        nf = Wn.shape[0] - 3 * D
        Wf = np.vstack([Wn[:nf], bn[None, :]]).astype(F16)
        Wh = np.ascontiguousarray(Wn[nf:nf + D]).astype(F16)
        ctxproj = (ctx_emb.astype(F32) @ Wn[nf + D:nf + 2 * D]).astype(F16)
        We = np.ascontiguousarray(Wn[nf + 2 * D:nf + 3 * D]).astype(F16)
        return Wf, Wh, ctxproj, We

    WfC, WhC, ctxprojC, WeC = node_w(W_c, b_c)
    WfV, WhV, ctxprojV, WeV = node_w(W_v, b_v)

    planA, edgesA = _side_plan(a_src, a_dst, cs, nv)
    planB, edgesB = _side_plan(c_src, c_dst, vs, ncl)
    nwinC, nwinV = planA["nwin"], planB["nwin"]

    per_core_maps = []
    for k in range(M):
        mA = _fill_side(planA, edgesA[k], inp["edge_sat_vc"], we_c, a_dst)
        mB = _fill_side(planB, edgesB[k], inp["edge_sat_cv"], we_v, c_dst)
        fTC, eTC, cxC, NpC = _node_prep(inp["clause_feats"][k * cs:(k + 1) * cs],
                                        clause_emb[k * cs:(k + 1) * cs],
                                        clause_ctx[k * cs:(k + 1) * cs], cs, nwinC)
        fTV, eTV, cxV, NpV = _node_prep(inp["var_feats"][k * vs:(k + 1) * vs],
                                        var_emb[k * vs:(k + 1) * vs],
                                        var_ctx[k * vs:(k + 1) * vs], vs, nwinV)
        per_core_maps.append(dict(
            idxA=mA["idxA"], dstA=mA["dstA"], wA=mA["wA"], satA=mA["satA"],
            idxB=mB["idxA"], dstB=mB["dstA"], wB=mB["wA"], satB=mB["satA"],
            featsTC=fTC, embTC=eTC, ctxC=cxC,
            featsTV=fTV, embTV=eTV, ctxV=cxV,
        ))

    repl_map = dict(
        gtabA=gtabA, gtabB=gtabB,
        WembA=WembA, WsatA=WsatA, WembB=WembB, WsatB=WsatB,
        WfC=WfC, WhC=WhC, WeC=WeC, ctxprojC=ctxprojC,
        WfV=WfV, WhV=WhV, WeV=WeV, ctxprojV=ctxprojV,
    )

    meta = dict(A=planA, B=planB, NpC=nwinC * WIN, NpV=nwinV * WIN,
                CS=cs, VS=vs, tabrows=dict(A=nv, B=ncl))
    nc = _build_program(meta)
    host = _run_spmd(nc, per_core_maps, repl_map)

    new_clause = host["outC"]                       # [8*cs, D] in core order
    new_var = host["outV"]                          # [8*vs, D]
    accC = host["accC"].reshape(M, P, 64).sum(0)    # [128 d, 64 u]
    accV = host["accV"].reshape(M, P, 64).sum(0)

    cnt_cu = np.bincount(clause_ctx, minlength=nu).astype(F32)
    cnt_vu = np.bincount(var_ctx, minlength=nu).astype(F32)
    c_ctx = (accC / np.maximum(cnt_cu, 1.0)[None, :]).T   # [64, 128]
    v_ctx = (accV / np.maximum(cnt_vu, 1.0)[None, :]).T
    zu = np.concatenate([inp["ctx_feats"].astype(F32), c_ctx, v_ctx,
                         ctx_emb.astype(F32)], 1) @ inp["W_u"].astype(F32) \
        + inp["b_u"].astype(F32)
    new_ctx = np.where(zu >= 0, zu, 0.1 * zu).astype(F32)

    return np.concatenate([new_clause, new_var, new_ctx], 0).astype(F32)


# revision 10
# speedup vs baseline: 10.7187x; 1.3849x over previous
"""Trainium2 Bass kernel for nn_MessageGNN (gnn_message_passing).

Sharding: destination-sharded edges across 8 cores.  Core k owns clauses
[k*50000,(k+1)*50000) and vars [k*12500,(k+1)*12500) and every edge whose
destination falls in its slice, so segment sums are fully core-local.

All 8 cores run ONE identical Bass program (SPMD) dispatched once via
shard_map — per-core variation lives entirely in the data.  The gather
schedule is made uniform by padding every (window, table-chunk) gather
group to the max count over the 8 cores (pad slots gather row 0, carry
zero sat/weight, dst sentinel 1536 so they contribute nothing).

Per core, per edge type:
  - Edges are laid out window-major (1024 destinations per window),
    bucketed by 32768-row gather-table chunk (int16 index limit of
    dma_gather) and sorted by destination.  x^T tiles arrive
    feature-major from fp16 transpose-mode dma_gather.
  - Edge MLP per 128-edge tile: stationary x^T / sat^T against moving
    weight chunks accumulate z[e,d] into a grouped PSUM tile; one Prelu
    (alpha=0.1) activation per 4-tile group does the leaky relu.
  - Segment-mean via one-hot matmul over the full 1024-dst window (one
    DVE op builds onehot * (1/cnt); two N=512 matmuls accumulate the
    window's h^T halves in PSUM).
  - Node MLP fused per window: 4 weight-chunk matmuls (feats+bias / h /
    ctx / emb) with the ctx gather folded into a host-computed
    projection driven by a one-hot.  Phase-3 partial sums accumulate
    into a [128,64] tile per node type; the 64-row ctx update finishes
    on host.
"""

import sys

sys.path.insert(0, "/opt/trn_rl_repo")

import numpy as np

NV, NC, NU, E, D = 100000, 400000, 64, 1200000, 128
M = 8
WIN = 1024
CHUNK = 32768
PAD_DST = 1536.0
P = 128
GRP = 4  # tiles per grouped-Prelu PSUM tile

F16 = np.float16
F32 = np.float32


def _wrap_idx(vals):
    n = len(vals)
    arr = np.zeros((16, n // 16), np.int16)
    if n:
        arr[np.arange(n) % 16, np.arange(n) // 16] = vals
    return np.tile(arr, (8, 1))


def _side_plan(src, dst, n_dst, tab_rows):
    """Uniform cross-core schedule for one edge type.

    Returns (sched, S, total_icols, percore) where sched is shared by all
    cores and percore[k] holds core k's sorted edge arrays."""
    nwin = (n_dst + WIN - 1) // WIN
    nchunk = (tab_rows + CHUNK - 1) // CHUNK
    counts = np.zeros((M, nwin, nchunk), np.int64)
    percore = []
    for k in range(M):
        base = k * n_dst
        mask = (dst >= base) & (dst < base + n_dst)
        es = np.nonzero(mask)[0]
        dstl = dst[es] - base
        srcl = src[es]
        w_id = dstl // WIN
        c_id = srcl // CHUNK
        order = np.lexsort((dstl, c_id, w_id))
        es, dstl, srcl, w_id, c_id = (a[order] for a in (es, dstl, srcl, w_id, c_id))
        np.add.at(counts[k], (w_id, c_id), 1)
        percore.append((es, dstl, srcl, w_id, c_id))
    npad = ((counts.max(0) + P - 1) // P) * P  # [nwin, nchunk]

    sched = []
    icol = 0
    for w in range(nwin):
        groups = []
        off = 0
        for c in range(nchunk):
            n = int(npad[w, c])
            if n == 0:
                continue
            groups.append(dict(chunk=c, n=n, off=off, icol=icol))
            off += n
            icol += n // 16
        sched.append(dict(slots=off, groups=groups))
    S = sum(wm["slots"] for wm in sched)
    if S == 0:
        sched[0] = dict(slots=P, groups=[dict(chunk=0, n=P, off=0, icol=0)])
        S, icol = P, P // 16
    return dict(sched=sched, S=S, icols=icol, nwin=nwin, nchunk=nchunk), percore


def _fill_side(plan, edges, sat, we, dst_glob):
    """Core-local slot arrays laid out per the shared schedule."""
    sched, S, icols, nchunk = plan["sched"], plan["S"], plan["icols"], plan["nchunk"]
    es, dstl, srcl, w_id, c_id = edges
    key = w_id * nchunk + c_id  # non-decreasing after the lexsort

    slot_src = np.zeros(S, np.int64)
    slot_dstw = np.full(S, -1, np.int64)
    slot_e = np.full(S, -1, np.int64)
    idxA = np.zeros((P, icols), np.int16)
    base = 0
    for w, wm in enumerate(sched):
        for g in wm["groups"]:
            c = g["chunk"]
            lo = np.searchsorted(key, w * nchunk + c, "left")
            hi = np.searchsorted(key, w * nchunk + c, "right")
            n = hi - lo
            s0 = base + g["off"]
            loc = np.zeros(g["n"], np.int64)
            loc[:n] = srcl[lo:hi] - c * CHUNK
            slot_src[s0:s0 + g["n"]] = loc
            slot_dstw[s0:s0 + n] = dstl[lo:hi] - w * WIN
            slot_e[s0:s0 + n] = es[lo:hi]
            idxA[:, g["icol"]:g["icol"] + g["n"] // 16] = _wrap_idx(loc)
        base += wm["slots"]

    dst_rel = np.where(slot_dstw >= 0, slot_dstw, int(PAD_DST)).astype(F32)
    real = slot_e >= 0
    wslot = np.zeros(S, F32)
    wslot[real] = we[dst_glob[slot_e[real]]]
    satA = np.zeros((5, S), F16)
    satA[:4, real] = sat[slot_e[real]].T.astype(F16)
    satA[4, real] = 1.0
    return dict(
        idxA=idxA,
        dstA=np.ascontiguousarray(dst_rel.reshape(S // P, P).T.astype(F32)),
        wA=np.ascontiguousarray(wslot.reshape(S // P, P).T.astype(F16)),
        satA=satA,
    )


def _node_prep(feats, emb, ctx_ids, n_nodes, nwin):
    Np = nwin * WIN
    fT = np.zeros((feats.shape[1] + 1, Np), F16)
    fT[:-1, :n_nodes] = feats.T.astype(F16)
    fT[-1, :n_nodes] = 1.0
    eT = np.zeros((P, Np), F16)
    eT[:, :n_nodes] = emb.T.astype(F16)
    cx = np.full(Np, 300.0, F32)
    cx[:n_nodes] = ctx_ids.astype(F32)
    cxT = np.ascontiguousarray(cx.reshape(Np // P, P).T.astype(F16))
    return fT, eT, cxT, Np


def _build_program(meta):
    import concourse.mybir as mybir
    import concourse.tile as tile
    from concourse import bacc
    from concourse.masks import make_identity

    f16, f32, i16, i32 = (mybir.dt.float16, mybir.dt.float32,
                          mybir.dt.int16, mybir.dt.int32)
    cs, vs = meta["CS"], meta["VS"]

    nc = bacc.Bacc("TRN2", target_bir_lowering=False, debug=False, num_devices=1)
    io = {}

    def dram(name, shape, dt, kind="ExternalInput"):
        io[name] = nc.dram_tensor(name, list(shape), dt, kind=kind)
        return io[name]

    for side in ("A", "B"):
        plan = meta[side]
        dram(f"gtab{side}", [meta["tabrows"][side], D], f16)
        dram(f"idx{side}", [P, plan["icols"]], i16)
        dram(f"dst{side}", [P, plan["S"] // P], f32)
        dram(f"w{side}", [P, plan["S"] // P], f16)
        dram(f"sat{side}", [5, plan["S"]], f16)
        dram(f"Wemb{side}", [P, D], f16)
        dram(f"Wsat{side}", [5, D], f16)
    for sd in ("C", "V"):
        Np = meta[f"Np{sd}"]
        dram(f"featsT{sd}", [17, Np], f16)
        dram(f"embT{sd}", [P, Np], f16)
        dram(f"ctx{sd}", [P, Np // P], f16)
        dram(f"Wf{sd}", [17, D], f16)
        dram(f"Wh{sd}", [P, D], f16)
        dram(f"We{sd}", [P, D], f16)
        dram(f"ctxproj{sd}", [64, D], f16)
    dram("outC", [cs, D], f16, kind="ExternalOutput")
    dram("outV", [vs, D], f16, kind="ExternalOutput")
    dram("accC", [P, 64], f32, kind="ExternalOutput")
    dram("accV", [P, 64], f32, kind="ExternalOutput")

    stage_max = max(
        max((wm["slots"] for wm in meta["A"]["sched"]), default=P),
        max((wm["slots"] for wm in meta["B"]["sched"]), default=P),
        P,
    )
    idx_max = max(
        max((g["n"] // 16 for plan in (meta["A"], meta["B"])
             for wm in plan["sched"] for g in wm["groups"]), default=8),
        8,
    )

    with tile.TileContext(nc) as tc:
        with tc.tile_pool(name="const", bufs=1) as cpool, \
             tc.tile_pool(name="stage", bufs=2) as spool, \
             tc.tile_pool(name="work", bufs=2) as wpool, \
             tc.tile_pool(name="hbuf", bufs=2) as hpool, \
             tc.tile_pool(name="psA", bufs=2, space="PSUM") as psA, \
             tc.tile_pool(name="psH", bufs=1, space="PSUM") as psH, \
             tc.tile_pool(name="psN", bufs=1, space="PSUM") as psN:

            ident = cpool.tile([P, P], f32)
            make_identity(nc, ident[:])
            identF = cpool.tile([P, P], f16)
            nc.vector.tensor_copy(identF[:], ident[:])
            iota_i = cpool.tile([P, WIN], i32)
            nc.gpsimd.iota(iota_i[:], pattern=[[1, WIN]], base=0, channel_multiplier=0)
            iota16 = cpool.tile([P, WIN], f16)
            nc.vector.tensor_copy(iota16[:], iota_i[:])
            iota64f = cpool.tile([P, 64], f32)
            nc.vector.tensor_copy(iota64f[:], iota_i[:, :64])
            z1 = cpool.tile([1, P], f16)
            nc.gpsimd.memset(z1[:], 0.0)
            z512 = cpool.tile([1, 512], f16)
            nc.gpsimd.memset(z512[:], 0.0)

            wt = {}
            for nm in ("WembA", "WsatA", "WembB", "WsatB",
                       "WfC", "WhC", "WeC", "ctxprojC",
                       "WfV", "WhV", "WeV", "ctxprojV"):
                t = cpool.tile(list(io[nm].shape), f16, tag=nm)
                nc.sync.dma_start(t[:], io[nm][:])
                wt[nm] = t

            acc_sb = {}
            for sd in ("C", "V"):
                a = cpool.tile([P, 64], f32, tag=f"acc{sd}")
                nc.vector.memset(a[:], 0.0)
                acc_sb[sd] = a

            for side, sd, n_nodes in (("A", "C", cs), ("B", "V", vs)):
                plan = meta[side]
                gtab = io[f"gtab{side}"]
                tabrows = meta["tabrows"][side]
                tile_off = 0
                for w, wm in enumerate(plan["sched"]):
                    slots = wm["slots"]
                    ntiles = slots // P
                    stage = spool.tile([P, 1, stage_max], f16, tag="stage")
                    for g in wm["groups"]:
                        n = g["n"]
                        it = wpool.tile([P, idx_max], i16, tag="idx")
                        nc.sync.dma_start(
                            it[:, :n // 16],
                            io[f"idx{side}"][:, g["icol"]:g["icol"] + n // 16])
                        c0 = g["chunk"] * CHUNK
                        c1 = min(c0 + CHUNK, tabrows)
                        # >512-idx transpose gathers crash the exec unit;
                        # split into <=512-idx calls (wrap layout slices
                        # cleanly at 512 = 32 idx columns)
                        for o in range(0, n, 512):
                            ns = min(512, n - o)
                            nc.gpsimd.dma_gather(
                                out_ap=stage[:, :, g["off"] + o:g["off"] + o + ns],
                                in_ap=gtab[c0:c1, :],
                                idxs_ap=it[:, o // 16:o // 16 + ns // 16],
                                num_idxs=ns, num_idxs_reg=ns, elem_size=D,
                                transpose=True)
                    if ntiles:
                        dstt = wpool.tile([P, max(ntiles, 1)], f32, tag="dstt")
                        nc.sync.dma_start(dstt[:, :ntiles],
                                          io[f"dst{side}"][:, tile_off:tile_off + ntiles])
                        wtt = wpool.tile([P, max(ntiles, 1)], f16, tag="wtt")
                        nc.sync.dma_start(wtt[:, :ntiles],
                                          io[f"w{side}"][:, tile_off:tile_off + ntiles])
                        satt = wpool.tile([5, stage_max], f16, tag="satt")
                        nc.sync.dma_start(
                            satt[:, :slots],
                            io[f"sat{side}"][:, tile_off * P:tile_off * P + slots])
                    hps = [psH.tile([P, 512], f32, tag=f"h{i}", name=f"hps{i}")
                           for i in range(2)]
                    for i in range(2):
                        nc.tensor.matmul(hps[i][:], lhsT=z1[:], rhs=z512[:],
                                         start=True, stop=False,
                                         skip_group_check=True)
                    for t0g in range(0, ntiles, GRP):
                        gn = min(GRP, ntiles - t0g)
                        mps = psA.tile([P, GRP * P], f32, tag="mps")
                        for j in range(gn):
                            t = t0g + j
                            nc.tensor.matmul(mps[:, j * P:(j + 1) * P],
                                             lhsT=stage[:, 0, t * P:(t + 1) * P],
                                             rhs=wt[f"Wemb{side}"][:],
                                             start=True, stop=False)
                            nc.tensor.matmul(mps[:, j * P:(j + 1) * P],
                                             lhsT=satt[:, t * P:(t + 1) * P],
                                             rhs=wt[f"Wsat{side}"][:],
                                             start=False, stop=True)
                        msb = wpool.tile([P, GRP * P], f16, tag="msb")
                        nc.scalar.activation(msb[:, :gn * P], mps[:, :gn * P],
                                             mybir.ActivationFunctionType.Prelu,
                                             alpha=0.1)
                        for j in range(gn):
                            t = t0g + j
                            ohw = wpool.tile([P, WIN], f16, tag="ohw")
                            nc.vector.scalar_tensor_tensor(
                                out=ohw[:], in0=iota16[:],
                                scalar=dstt[:, t:t + 1],
                                in1=wtt[:, t:t + 1].to_broadcast([P, WIN]),
                                op0=mybir.AluOpType.is_equal,
                                op1=mybir.AluOpType.mult)
                            for half in range(2):
                                nc.tensor.matmul(
                                    hps[half][:], lhsT=msb[:, j * P:(j + 1) * P],
                                    rhs=ohw[:, half * 512:(half + 1) * 512],
                                    start=False, stop=True, skip_group_check=True)
                    tile_off += ntiles
                    hT = hpool.tile([P, WIN], f16, tag="hT")
                    nc.vector.tensor_copy(hT[:, :512], hps[0][:])
                    nc.vector.tensor_copy(hT[:, 512:], hps[1][:])

                    # ---- node phase for this window (WIN nodes, padded) ----
                    for g0 in (0, 512):
                        cga = w * WIN + g0
                        ctx16 = wpool.tile([P, 4], f16, tag="ctx16")
                        nc.sync.dma_start(ctx16[:], io[f"ctx{sd}"][:, cga // P:cga // P + 4])
                        ctx32 = wpool.tile([P, 4], f32, tag="ctx32")
                        nc.vector.tensor_copy(ctx32[:], ctx16[:])
                        featsl = wpool.tile([17, 512], f16, tag="featsl")
                        nc.sync.dma_start(featsl[:], io[f"featsT{sd}"][:, cga:cga + 512])
                        embl = wpool.tile([P, 512], f16, tag="embl")
                        nc.sync.dma_start(embl[:], io[f"embT{sd}"][:, cga:cga + 512])
                        ohuT = wpool.tile([64, 512], f16, tag="ohuT")
                        ohu_f = []
                        for j in range(4):
                            ohuf = wpool.tile([P, 64], f16, tag=f"ohuf{j}")
                            nc.vector.tensor_single_scalar(
                                out=ohuf[:], in_=iota64f[:],
                                scalar=ctx32[:, j:j + 1], op=mybir.AluOpType.is_equal)
                            ohu_f.append(ohuf)
                            tps = psA.tile([P, P], f16, tag="tp")
                            nc.tensor.matmul(tps[:64, :], lhsT=ohuf[:], rhs=identF[:],
                                             is_transpose=True, skip_group_check=True)
                            nc.vector.tensor_copy(ohuT[:, j * P:(j + 1) * P], tps[:64, :])
                        nps = psN.tile([P, 512], f32, tag="nps")
                        nc.tensor.matmul(nps[:], lhsT=wt[f"Wf{sd}"][:],
                                         rhs=featsl[:], start=True, stop=False)
                        nc.tensor.matmul(nps[:], lhsT=wt[f"Wh{sd}"][:],
                                         rhs=hT[:, g0:g0 + 512], start=False, stop=False)
                        nc.tensor.matmul(nps[:], lhsT=wt[f"ctxproj{sd}"][:],
                                         rhs=ohuT[:], start=False, stop=False)
                        nc.tensor.matmul(nps[:], lhsT=wt[f"We{sd}"][:],
                                         rhs=embl[:], start=False, stop=True)
                        nsb = wpool.tile([P, 512], f16, tag="nsb")
                        nc.scalar.activation(nsb[:], nps[:],
                                             mybir.ActivationFunctionType.Prelu,
                                             alpha=0.1)
                        aps = psN.tile([P, 64], f32, tag="aps")
                        for j in range(4):
                            rows = min(P, max(0, n_nodes - (cga + j * P)))
                            tps2 = psA.tile([P, P], f16, tag="tp")
                            nc.tensor.matmul(tps2[:], lhsT=nsb[:, j * P:(j + 1) * P],
                                             rhs=identF[:], is_transpose=True,
                                             skip_group_check=True)
                            osb = wpool.tile([P, P], f16, tag="osb")
                            nc.vector.tensor_copy(osb[:], tps2[:])
                            if rows > 0:
                                out_t = io["outC"] if sd == "C" else io["outV"]
                                nc.sync.dma_start(
                                    out_t[cga + j * P:cga + j * P + rows, :],
                                    osb[:rows, :])
                            nc.tensor.matmul(aps[:], lhsT=osb[:], rhs=ohu_f[j][:],
                                             start=(j == 0), stop=(j == 3))
                        nc.vector.tensor_add(acc_sb[sd][:], acc_sb[sd][:], aps[:])

            nc.sync.dma_start(io["accC"][:], acc_sb["C"][:])
            nc.sync.dma_start(io["accV"][:], acc_sb["V"][:])
    nc.compile()
    return nc


_spmd_state = {}

REPLICATED = ("gtabA", "gtabB", "WembA", "WsatA", "WembB", "WsatB",
              "WfC", "WhC", "WeC", "ctxprojC", "WfV", "WhV", "WeV", "ctxprojV")


def _run_spmd(nc, per_core_maps, repl_map):
    """One shard_map dispatch running the identical program on all 8 cores."""
    import concourse.mybir as mybir
    import jax
    from concourse.bass2jax import (_bass_exec_p, install_neuronx_cc_hook,
                                    partition_id_tensor)
    from jax.experimental.shard_map import shard_map
    from jax.sharding import Mesh, NamedSharding, PartitionSpec

    install_neuronx_cc_hook()
    partition_name = nc.partition_id_tensor.name if nc.partition_id_tensor else None
    in_names, out_names, out_avals, zero_shapes = [], [], [], []
    for alloc in nc.m.functions[0].allocations:
        if not isinstance(alloc, mybir.MemoryLocationSet):
            continue
        name = alloc.memorylocations[0].name
        if alloc.kind == "ExternalInput":
            if name != partition_name:
                in_names.append(name)
        elif alloc.kind == "ExternalOutput":
            shape = tuple(alloc.tensor_shape)
            dtype = mybir.dt.np(alloc.dtype)
            out_names.append(name)
            out_avals.append(jax.core.ShapedArray(shape, dtype))
            zero_shapes.append((shape, dtype))
    n_params = len(in_names)
    n_outs = len(out_names)
    all_names = list(in_names) + list(out_names)
    if partition_name is not None:
        all_names.append(partition_name)
    donate = tuple(range(n_params, n_params + n_outs))

    def _body(*args):
        operands = list(args)
        if partition_name is not None:
            operands.append(partition_id_tensor())
        return tuple(_bass_exec_p.bind(
            *operands, out_avals=tuple(out_avals), in_names=tuple(all_names),
            out_names=tuple(out_names), lowering_input_output_aliases=(),
            sim_require_finite=True, sim_require_nnan=True, nc=nc))

    devices = jax.devices()[:M]
    mesh = Mesh(np.asarray(devices), ("core",))
    in_specs = tuple(
        PartitionSpec() if nm in REPLICATED else PartitionSpec("core")
        for nm in in_names) + (PartitionSpec("core"),) * n_outs
    out_specs = (PartitionSpec("core"),) * n_outs
    sharded = jax.jit(
        shard_map(_body, mesh=mesh, in_specs=in_specs, out_specs=out_specs,
                  check_rep=False),
        donate_argnums=donate, keep_unused=True)

    sh_core = NamedSharding(mesh, PartitionSpec("core"))
    sh_repl = NamedSharding(mesh, PartitionSpec())

    # Big replicated tables ship sharded (1x over the wire) and are
    # broadcast device-side by an all_gather; small ones ship replicated.
    def _ag(x):
        return jax.lax.all_gather(x, "core", axis=0, tiled=True)

    ag_fn = jax.jit(
        shard_map(_ag, mesh=mesh, in_specs=(PartitionSpec("core"),),
                  out_specs=PartitionSpec(), check_rep=False),
        out_shardings=sh_repl)

    d_ins = []
    for nm in in_names:
        if nm in REPLICATED:
            a = repl_map[nm]
            if a.nbytes > 8 << 20 and a.shape[0] % M == 0:
                d_ins.append(ag_fn(jax.device_put(a, sh_core)))
            else:
                d_ins.append(jax.device_put(a, sh_repl))
        else:
            d_ins.append(jax.device_put(
                np.concatenate([m[nm] for m in per_core_maps], 0), sh_core))
    d_zeros = [jax.device_put(np.zeros((M * s[0], *s[1:]), dt), sh_core)
               for (s, dt) in zero_shapes]

    outs = sharded(*d_ins, *d_zeros)
    host = {nm: np.asarray(o) for nm, o in zip(out_names, outs)}

    _spmd_state.clear()
    _spmd_state.update(fn=sharded, d_ins=d_ins, outs=list(outs),
                       out_names=out_names)
    return host


def kernel(**inputs):
    inp = {k: np.asarray(v) for k, v in inputs.items()}
    var_emb, clause_emb, ctx_emb = inp["var_emb"], inp["clause_emb"], inp["ctx_emb"]
    nv, ncl, nu = var_emb.shape[0], clause_emb.shape[0], ctx_emb.shape[0]
    cs, vs = ncl // M, nv // M

    W_vc, b_vc = inp["W_vc"].astype(F32), inp["b_vc"].astype(F32)
    W_cv, b_cv = inp["W_cv"].astype(F32), inp["b_cv"].astype(F32)
    W_c, b_c = inp["W_c"].astype(F32), inp["b_c"].astype(F32)
    W_v, b_v = inp["W_v"].astype(F32), inp["b_v"].astype(F32)

    a_src = inp["assigns_src"].astype(np.int64)
    a_dst = inp["assigns_dst"].astype(np.int64)
    c_src = inp["contains_src"].astype(np.int64)
    c_dst = inp["contains_dst"].astype(np.int64)
    var_ctx = inp["var_ctx"].astype(np.int64)
    clause_ctx = inp["clause_ctx"].astype(np.int64)

    cnt_c = np.bincount(a_dst, minlength=ncl).astype(F32)
    cnt_v = np.bincount(c_dst, minlength=nv).astype(F32)
    we_c = 1.0 / np.maximum(cnt_c, 1.0)
    we_v = 1.0 / np.maximum(cnt_v, 1.0)

    gtabA = var_emb.astype(F16)      # assigns gathers var_emb
    gtabB = clause_emb.astype(F16)   # contains gathers clause_emb

    # edge MLP weight chunks (+bias row on the sat chunk)
    WembA = np.ascontiguousarray(W_vc[4:4 + D]).astype(F16)
    WsatA = np.vstack([W_vc[:4], b_vc[None, :]]).astype(F16)
    WembB = np.ascontiguousarray(W_cv[4:4 + D]).astype(F16)
    WsatB = np.vstack([W_cv[:4], b_cv[None, :]]).astype(F16)

    # node MLP chunks: rows [0:16 feats][16:144 h][144:272 ctx][272:400 emb]
    def node_w(Wn, bn):
        nf = Wn.shape[0] - 3 * D
        Wf = np.vstack([Wn[:nf], bn[None, :]]).astype(F16)
        Wh = np.ascontiguousarray(Wn[nf:nf + D]).astype(F16)
        ctxproj = (ctx_emb.astype(F32) @ Wn[nf + D:nf + 2 * D]).astype(F16)
        We = np.ascontiguousarray(Wn[nf + 2 * D:nf + 3 * D]).astype(F16)
        return Wf, Wh, ctxproj, We

    WfC, WhC, ctxprojC, WeC = node_w(W_c, b_c)
    WfV, WhV, ctxprojV, WeV = node_w(W_v, b_v)

    planA, edgesA = _side_plan(a_src, a_dst, cs, nv)
    planB, edgesB = _side_plan(c_src, c_dst, vs, ncl)
    nwinC, nwinV = planA["nwin"], planB["nwin"]

    per_core_maps = []
    for k in range(M):
        mA = _fill_side(planA, edgesA[k], inp["edge_sat_vc"], we_c, a_dst)
        mB = _fill_side(planB, edgesB[k], inp["edge_sat_cv"], we_v, c_dst)
        fTC, eTC, cxC, NpC = _node_prep(inp["clause_feats"][k * cs:(k + 1) * cs],
                                        clause_emb[k * cs:(k + 1) * cs],
                                        clause_ctx[k * cs:(k + 1) * cs], cs, nwinC)
        fTV, eTV, cxV, NpV = _node_prep(inp["var_feats"][k * vs:(k + 1) * vs],
                                        var_emb[k * vs:(k + 1) * vs],
                                        var_ctx[k * vs:(k + 1) * vs], vs, nwinV)
        per_core_maps.append(dict(
            idxA=mA["idxA"], dstA=mA["dstA"], wA=mA["wA"], satA=mA["satA"],
            idxB=mB["idxA"], dstB=mB["dstA"], wB=mB["wA"], satB=mB["satA"],
            featsTC=fTC, embTC=eTC, ctxC=cxC,
            featsTV=fTV, embTV=eTV, ctxV=cxV,
        ))

    repl_map = dict(
        gtabA=gtabA, gtabB=gtabB,
        WembA=WembA, WsatA=WsatA, WembB=WembB, WsatB=WsatB,
        WfC=WfC, WhC=WhC, WeC=WeC, ctxprojC=ctxprojC,
        WfV=WfV, WhV=WhV, WeV=WeV, ctxprojV=ctxprojV,
    )

    meta = dict(A=planA, B=planB, NpC=nwinC * WIN, NpV=nwinV * WIN,
                CS=cs, VS=vs, tabrows=dict(A=nv, B=ncl))
    nc = _build_program(meta)
    host = _run_spmd(nc, per_core_maps, repl_map)

    new_clause = host["outC"].astype(F32)           # [8*cs, D] in core order
    new_var = host["outV"].astype(F32)              # [8*vs, D]
    accC = host["accC"].reshape(M, P, 64).sum(0)    # [128 d, 64 u]
    accV = host["accV"].reshape(M, P, 64).sum(0)

    cnt_cu = np.bincount(clause_ctx, minlength=nu).astype(F32)
    cnt_vu = np.bincount(var_ctx, minlength=nu).astype(F32)
    c_ctx = (accC / np.maximum(cnt_cu, 1.0)[None, :]).T   # [64, 128]
    v_ctx = (accV / np.maximum(cnt_vu, 1.0)[None, :]).T
    zu = np.concatenate([inp["ctx_feats"].astype(F32), c_ctx, v_ctx,
                         ctx_emb.astype(F32)], 1) @ inp["W_u"].astype(F32) \
        + inp["b_u"].astype(F32)
    new_ctx = np.where(zu >= 0, zu, 0.1 * zu).astype(F32)

    return np.concatenate([new_clause, new_var, new_ctx], 0).astype(F32)
